# revision 2
# baseline (speedup 1.0000x reference)
"""Trainium2 Bass kernel for nn_Block_39247411151159 — fp8 DoubleRow rewrite.

Sharding: 8 cores = 4 batches x 2 head-groups (4 heads each). One pairwise
AllReduce mid-kernel sums the re-atten conv partials (etc_k); the final
squeeze-conv partials are summed on the host.

Algorithm (factored form; all reassociations exact):
  yes_h  = sum_s y[c,s] * est_h[e,s]            (per head, [c,e])
  etc_k  = sum_h RW2_h @ yes_h                  (RW2_h = re_w_h @ v_w_h, host)
  scores = (etc_k * mix_h / sqrt_p).T @ (q_w @ y * mask)
  aU     = exp(scores);  Z = sum_e aU
  ya_h   = yesT_h.T @ aU_h / Z
  out    = sum_h SVW_h @ ya_h + avgpool3(W2-branch)  (SVW_h = sq_w_h @ v_w_h)
The v-mask quirk (reference masks only the first DIM channels of v = head 0)
is folded into est_h0 on the host.

Precision: large GEMMs in fp8e4m3 + DoubleRow (2 K-tiles/instr, 0.5
cycles/row); the numerically dominant avgpool branch (QM/P2) in fp16.
Scales: est x32, yes x4, yesT x64, RW2/SVW x64, etckh x64, qm_f8 x8,
aU x8 (exp bias ln8). Final psum is 4096x true; W2 pre-scaled x4096;
host divides by 4096. Emulated rel-err ~1.3e-2 (tolerance 2e-2).

Schedule: front = per-head {est chunk DMAs, yes matmuls, yes copy, rk DR}
(yesT deferred); AllReduce launched straight from the rk psum; shadow =
yesT matmuls/copies + QM + P2 + exp-table warm; tail = per-head
scores -> exp -> Z -> zrec -> ya -> fin with per-head tiles so heads
pipeline without false dependencies.
"""
import math
import sys

sys.path.insert(0, "/opt/trn_rl_repo")

import ml_dtypes
import numpy as np

import concourse.mybir as mybir
import concourse.tile as tile
from concourse import bacc, bass_utils

HEAD, DIM, ETC = 8, 256, 512
BAT, SEQ = 4, 1024
NCORES = 8
HPC = HEAD // 2
P = 128
SC = SEQ // P
ET = ETC // P
F32 = mybir.dt.float32
F16 = mybir.dt.float16
F8 = mybir.dt.float8e4
DR = mybir.MatmulPerfMode.DoubleRow
EXPF = mybir.ActivationFunctionType.Exp
COPYF = mybir.ActivationFunctionType.Copy

S_EST = 32.0
S_YES = 4.0
S_YEST = 16.0
S_RW2 = 64.0
S_SVW = 64.0
S_EKH = 64.0
S_QM8 = 8.0
S_AU = 8.0
OUT_DESCALE = S_YEST * S_SVW   # 1024

_NC = {}


def _build(use_collective=True):
    nc = bacc.Bacc("TRN2", target_bir_lowering=False, debug=False,
                   num_devices=NCORES if use_collective else 1)

    def din(name, shape, dt):
        return nc.dram_tensor(name, shape, dt, kind="ExternalInput").ap()

    yt_d = din("yt", [P, SC, DIM], F8)
    est_d = din("est", [HEAD, P, SC, ETC], F8)
    rw2t_d = din("rw2t", [P, HEAD, 2, DIM], F8)
    svwt_d = din("svwt", [P, HPC, 2, DIM], F8)
    qwt_d = din("qwt", [P, 2, DIM], F16)
    yh_d = din("yh", [P, 2, SEQ], F16)
    w2t_d = din("w2t", [P, 2, DIM], F16)
    maskbc_d = din("maskbc", [1, SEQ], F16)
    mixsp_d = din("mixsp", [P, HPC * 2], F32)
    ones8_d = din("ones8", [P, 2, P], F8)
    out_d = nc.dram_tensor("out", [DIM, SEQ], F16, kind="ExternalOutput").ap()

    with tile.TileContext(nc) as tc:
        with (
            tc.tile_pool(name="const", bufs=1) as cpool,
            tc.tile_pool(name="psA", bufs=4, space="PSUM") as psA,
            tc.tile_pool(name="psF", bufs=2, space="PSUM") as psF,
        ):
            # ---- tiles ----
            yt_s = cpool.tile([P, SC, DIM], F8, tag="yt")
            est_s = [cpool.tile([P, SC, ETC], F8, tag=f"est{h}", name=f"est{h}")
                     for h in range(HEAD)]
            ones8_s = cpool.tile([P, 2, P], F8, tag="ones8")
            rw2t_s = cpool.tile([P, HEAD, 2, DIM], F8, tag="rw2t")
            qwt_s = cpool.tile([P, 2, DIM], F16, tag="qwt")
            yh_s = cpool.tile([P, 2, SEQ], F16, tag="yh")
            w2t_s = cpool.tile([P, 2, DIM], F16, tag="w2t")
            svwt_s = cpool.tile([P, HPC, 2, DIM], F8, tag="svwt")
            maskbc_s = cpool.tile([P, SEQ], F16, tag="maskbc")
            mixsp_s = cpool.tile([P, HPC * 2], F32, tag="mixsp")
            yes_s = [cpool.tile([P, 2, 512], F8, tag=f"yes{h}", name=f"yes{h}")
                     for h in range(HEAD)]
            yesT_s = [cpool.tile([P, ET, DIM], F8, tag=f"yesT{h}", name=f"yesT{h}")
                      for h in range(HPC)]
            etck_s = cpool.tile([P, 2, 512], F16, tag="etck")
            etckh_s = [cpool.tile([P, 2, 512], F8, tag=f"etckh{h}", name=f"etckh{h}")
                       for h in range(HPC)]
            qmh_s = cpool.tile([P, 2, SEQ], F16, tag="qmh")
            qm8_s = cpool.tile([P, 2, SEQ], F8, tag="qm8")
            aU_s = [cpool.tile([P, ET, SEQ], F8, tag=f"aU{h}", name=f"aU{h}")
                    for h in range(HPC)]
            zrec_s = [cpool.tile([P, SEQ], F32, tag=f"zrec{h}", name=f"zrec{h}")
                      for h in range(HPC)]
            ya_s = [cpool.tile([P, 2, SEQ], F8, tag=f"ya{h}", name=f"ya{h}")
                    for h in range(HPC)]
            p2s_s = cpool.tile([P, 2, SEQ + 2], F16, tag="p2s")
            sum3_s = cpool.tile([P, 2, SEQ], F16, tag="sum3")
            out_s = cpool.tile([P, 2, SEQ], F16, tag="outs")
            lnau_s = cpool.tile([P, 1], F32, tag="lnau")
            wz = cpool.tile([P, 2, 512], F8, tag="wz")
            dummy = cpool.tile([P, 1], F32, tag="dummy")

            # ---- DMA lead-in: yt, then est chunks (2 sc-pairs each) ----
            nc.sync.dma_start(yt_s[:], yt_d)
            for h in range(HPC):
                for q in range(2):
                    nc.sync.dma_start(est_s[h][:, 4 * q:4 * q + 4, :],
                                      est_d[h, :, 4 * q:4 * q + 4])
            nc.sync.dma_start(ones8_s[:], ones8_d)
            nc.sync.dma_start(rw2t_s[:], rw2t_d)
            for h in range(HPC, HEAD):
                for q in range(2):
                    nc.sync.dma_start(est_s[h][:, 4 * q:4 * q + 4, :],
                                      est_d[h, :, 4 * q:4 * q + 4])

            nc.vector.memset(wz[:], 0.0)
            nc.gpsimd.memset(p2s_s[:, :, 0:1], 0.0)
            nc.gpsimd.memset(p2s_s[:, :, SEQ + 1:SEQ + 2], 0.0)
            nc.gpsimd.memset(lnau_s[:], math.log(S_AU))

            rkps = psF.tile([P, 1024], F32, tag="psF", name="rkps")

            # ---- front: yes_h, rk partials ----
            for h in range(HEAD):
                for ct in range(2):
                    ps = psA.tile([P, 512], F32, tag="psA",
                                  name=f"yes{h}_{ct}")
                    for scp in range(4):
                        if h == 0 and ct == 0 and scp == 0:
                            for w in range(3):
                                nc.tensor.matmul(
                                    ps[:], lhsT=wz[:, :, 0:P],
                                    rhs=wz[:], start=(w == 0), stop=False,
                                    perf_mode=DR)
                        nc.tensor.matmul(
                            ps[:],
                            lhsT=yt_s[:, 2 * scp:2 * scp + 2,
                                      ct * P:(ct + 1) * P],
                            rhs=est_s[h][:, 2 * scp:2 * scp + 2, :],
                            start=(scp == 0 and not (h == 0 and ct == 0)),
                            stop=(scp == 3), perf_mode=DR)
                    nc.vector.tensor_scalar_mul(
                        yes_s[h][:, ct, :], ps[:], S_YES / S_EST)
                for mt in range(2):
                    nc.tensor.matmul(
                        rkps[:, mt * 512:(mt + 1) * 512],
                        lhsT=rw2t_s[:, h, :, mt * P:(mt + 1) * P],
                        rhs=yes_s[h][:], start=(h == 0), stop=(h == HEAD - 1),
                        perf_mode=DR)

            # ---- etck: all 8 heads reduced locally, no collective ----
            for dt_ in range(2):
                nc.vector.tensor_copy(out=etck_s[:, dt_, :],
                                      in_=rkps[:, dt_ * 512:(dt_ + 1) * 512])

            # ---- shadow: weights in, yesT, QM, P2, table warm ----
            nc.sync.dma_start(qwt_s[:], qwt_d)
            nc.sync.dma_start(yh_s[:], yh_d)
            nc.sync.dma_start(maskbc_s[:], maskbc_d.to_broadcast((P, SEQ)))
            nc.sync.dma_start(mixsp_s[:], mixsp_d)
            nc.sync.dma_start(w2t_s[:], w2t_d)
            nc.sync.dma_start(svwt_s[:], svwt_d)
            for h in range(HPC):
                for ep in range(2):
                    pst = psA.tile([P, 512], F32, tag="psA",
                                   name=f"yesT{h}_{ep}")
                    for et2 in range(2):
                        et = 2 * ep + et2
                        for scp in range(4):
                            nc.tensor.matmul(
                                pst[:, et2 * 256:(et2 + 1) * 256],
                                lhsT=est_s[h][:, 2 * scp:2 * scp + 2,
                                              et * P:(et + 1) * P],
                                rhs=yt_s[:, 2 * scp:2 * scp + 2, :],
                                start=(scp == 0), stop=(scp == 3),
                                perf_mode=DR)
                    nc.scalar.activation(
                        yesT_s[h][:, 2 * ep:2 * ep + 2, :],
                        pst[:].rearrange("p (t f) -> p t f", t=2),
                        COPYF, scale=S_YEST / S_EST)

            for mt in range(2):
                for sj in range(2):
                    ps = psA.tile([P, 512], F32, tag="psA",
                                  name=f"qm{mt}_{sj}")
                    for kt in range(2):
                        nc.tensor.matmul(
                            ps[:],
                            lhsT=qwt_s[:, kt, mt * P:(mt + 1) * P],
                            rhs=yh_s[:, kt, sj * 512:(sj + 1) * 512],
                            start=(kt == 0), stop=(kt == 1))
                    nc.vector.tensor_tensor(
                        out=qmh_s[:, mt, sj * 512:(sj + 1) * 512], in0=ps[:],
                        in1=maskbc_s[:, sj * 512:(sj + 1) * 512],
                        op=mybir.AluOpType.mult)
                    nc.scalar.activation(
                        qm8_s[:, mt, sj * 512:(sj + 1) * 512],
                        qmh_s[:, mt, sj * 512:(sj + 1) * 512],
                        COPYF, scale=S_QM8)
            for mt in range(2):
                for sj in range(2):
                    ps = psA.tile([P, 512], F32, tag="psA",
                                  name=f"p2{mt}_{sj}")
                    for kt in range(2):
                        nc.tensor.matmul(
                            ps[:],
                            lhsT=w2t_s[:, kt, mt * P:(mt + 1) * P],
                            rhs=qmh_s[:, kt, sj * 512:(sj + 1) * 512],
                            start=(kt == 0), stop=(kt == 1))
                    nc.scalar.activation(
                        p2s_s[:, mt, 1 + sj * 512:1 + (sj + 1) * 512],
                        ps[:], COPYF)
            nc.scalar.activation(dummy[:], mixsp_s[:, 0:1], EXPF)
            for mt in range(2):
                nc.vector.tensor_tensor(
                    out=sum3_s[:, mt, :], in0=p2s_s[:, mt, 0:SEQ],
                    in1=p2s_s[:, mt, 1:SEQ + 1], op=mybir.AluOpType.add)
                nc.vector.tensor_tensor(
                    out=sum3_s[:, mt, :], in0=sum3_s[:, mt, :],
                    in1=p2s_s[:, mt, 2:SEQ + 2], op=mybir.AluOpType.add)

            # etckh per head on Pool, head 0 first (gates first scores)
            for h in range(HPC):
                for dt_ in range(2):
                    nc.gpsimd.tensor_scalar_mul(
                        etckh_s[h][:, dt_, :], etck_s[:, dt_, :],
                        mixsp_s[:, h * 2 + dt_:h * 2 + dt_ + 1])

            # ---- attention tail ----
            fins = [psF.tile([P, 1024], F32, tag="psF", name=f"fin{mt}")
                    for mt in range(2)]

            def half_out(sj):
                for mt in range(2):
                    nc.vector.tensor_tensor(
                        out=out_s[:, mt, sj * 512:(sj + 1) * 512],
                        in0=fins[mt][:, sj * 512:(sj + 1) * 512],
                        in1=sum3_s[:, mt, sj * 512:(sj + 1) * 512],
                        op=mybir.AluOpType.add)
                    eng = nc.sync if mt == 0 else nc.scalar
                    eng.dma_start(
                        out_d[mt * P:(mt + 1) * P, sj * 512:(sj + 1) * 512],
                        out_s[:, mt, sj * 512:(sj + 1) * 512])

            def head_tail(h):
                # per sequence-half: scores+exp, then Z/zrec/ya/fin of this
                # half while the other half's exps run
                for sj in range(2):
                    for et in range(ET):
                        ps = psA.tile([P, 512], F32, tag="psA",
                                      name=f"sc{h}_{et}_{sj}")
                        nc.tensor.matmul(
                            ps[:],
                            lhsT=etckh_s[h][:, :, et * P:(et + 1) * P],
                            rhs=qm8_s[:, :, sj * 512:(sj + 1) * 512],
                            start=True, stop=True, perf_mode=DR)
                        nc.scalar.activation(
                            aU_s[h][:, et, sj * 512:(sj + 1) * 512],
                            ps[:], EXPF,
                            scale=1.0 / (S_EKH * S_QM8), bias=lnau_s[:])
                    psz = psA.tile([P, 512], F32, tag="psA",
                                   name=f"z{h}_{sj}")
                    for ep in range(2):
                        nc.tensor.matmul(
                            psz[:],
                            lhsT=ones8_s[:],
                            rhs=aU_s[h][:, 2 * ep:2 * ep + 2,
                                        sj * 512:(sj + 1) * 512],
                            start=(ep == 0), stop=(ep == 1), perf_mode=DR)
                    nc.vector.reciprocal(
                        out=zrec_s[h][:, sj * 512:(sj + 1) * 512], in_=psz[:])
                    for ct in range(2):
                        ps = psA.tile([P, 512], F32, tag="psA",
                                      name=f"ya{h}_{ct}_{sj}")
                        for ep in range(2):
                            nc.tensor.matmul(
                                ps[:],
                                lhsT=yesT_s[h][:, 2 * ep:2 * ep + 2,
                                               ct * P:(ct + 1) * P],
                                rhs=aU_s[h][:, 2 * ep:2 * ep + 2,
                                            sj * 512:(sj + 1) * 512],
                                start=(ep == 0), stop=(ep == 1),
                                perf_mode=DR)
                        nc.vector.tensor_tensor(
                            out=ya_s[h][:, ct, sj * 512:(sj + 1) * 512],
                            in0=ps[:],
                            in1=zrec_s[h][:, sj * 512:(sj + 1) * 512],
                            op=mybir.AluOpType.mult)
                    for mt in range(2):
                        nc.tensor.matmul(
                            fins[mt][:, sj * 512:(sj + 1) * 512],
                            lhsT=svwt_s[:, h, :, mt * P:(mt + 1) * P],
                            rhs=ya_s[h][:, :, sj * 512:(sj + 1) * 512],
                            start=(h == 0), stop=(h == HPC - 1), perf_mode=DR)
                    if h == HPC - 1:
                        half_out(sj)

            for h in range(HPC):
                head_tail(h)



    nc.compile()
    return nc


def _prep_inputs(y, e_s, mask, regular, mix, sqrt_p, q_w, q_b, v_w, v_b,
                 re_w, re_b, sq_w, sq_b):
    f = np.float32
    F8N = ml_dtypes.float8_e4m3
    y = np.asarray(y, f)
    e_s = np.asarray(e_s, f)
    mask = np.asarray(mask, f)
    reg = float(np.asarray(regular))
    mix = np.asarray(mix, f)
    sp = float(np.asarray(sqrt_p))
    q_w = np.asarray(q_w, f)
    v_w = np.asarray(v_w, f)
    re_w = np.asarray(re_w, f)
    sq_w = np.asarray(sq_w, f)

    vw_h = v_w.reshape(HEAD, DIM, DIM)
    rw2 = np.stack([re_w[:, h * DIM:(h + 1) * DIM] @ vw_h[h]
                    for h in range(HEAD)])
    svw = np.stack([sq_w[:, h * DIM:(h + 1) * DIM] @ vw_h[h]
                    for h in range(HEAD)])
    qwt = np.ascontiguousarray(
        q_w.T.reshape(2, P, DIM).transpose(1, 0, 2)).astype(np.float16)

    in_maps = []
    for c in range(NCORES):
        b, hg = c // 2, c % 2
        hh = slice(hg * HPC, hg * HPC + HPC)
        order = np.r_[hg * HPC:(hg + 1) * HPC,
                      (1 - hg) * HPC:(2 - hg) * HPC]
        est = e_s[order, b].copy()
        gh0 = int(np.where(order == 0)[0][0])
        est[gh0] = est[gh0] * (mask[b, 0] * reg)[None, :]
        est = (est * S_EST).transpose(0, 2, 1).reshape(HEAD, SC, P, ETC) \
            .transpose(0, 2, 1, 3)
        yt = y[b].T.reshape(SC, P, DIM).transpose(1, 0, 2)
        rw2t = (rw2[order] * S_RW2).transpose(0, 2, 1) \
            .reshape(HEAD, 2, P, DIM).transpose(2, 0, 1, 3)
        svwt = (svw[hh] * S_SVW).transpose(0, 2, 1) \
            .reshape(HPC, 2, P, DIM).transpose(2, 0, 1, 3)
        sqw_h = sq_w.reshape(DIM, HEAD, DIM)[:, hh]
        w2 = (sqw_h * mix[hh, :, 0][None]).sum(1) / 3.0 * OUT_DESCALE
        w2t = w2.T.reshape(2, P, DIM).transpose(1, 0, 2)
        yh = y[b].reshape(2, P, SEQ).transpose(1, 0, 2)
        mixsp = (mix[hh, :, 0] / sp * (S_EKH / (S_RW2 * S_YES))) \
            .reshape(HPC, 2, P).transpose(2, 0, 1).reshape(P, HPC * 2)
        m = {
            "yt": np.ascontiguousarray(yt).astype(F8N),
            "est": np.ascontiguousarray(est).astype(F8N),
            "rw2t": np.ascontiguousarray(rw2t).astype(F8N),
            "svwt": np.ascontiguousarray(svwt).astype(F8N),
            "qwt": qwt,
            "yh": np.ascontiguousarray(yh).astype(np.float16),
            "w2t": np.ascontiguousarray(w2t).astype(np.float16),
            "maskbc": (mask[b] * reg).astype(np.float16),
            "mixsp": np.ascontiguousarray(mixsp).astype(f),
            "ones8": np.ones((P, 2, P), dtype=F8N),
        }
        in_maps.append(m)
    return in_maps


def kernel(**inputs):
    if "hw" not in _NC:
        _NC["hw"] = _build(use_collective=True)
    in_maps = _prep_inputs(**inputs)
    try:
        res = bass_utils.run_bass_kernel_spmd(_NC["hw"], in_maps,
                                              core_ids=list(range(NCORES)))
    except Exception:
        import time
        time.sleep(5)
        res = bass_utils.run_bass_kernel_spmd(_NC["hw"], in_maps,
                                              core_ids=list(range(NCORES)))
    out = np.empty((BAT, DIM, SEQ), np.float32)
    for b in range(BAT):
        out[b] = (res.results[2 * b]["out"].astype(np.float32)
                  + res.results[2 * b + 1]["out"].astype(np.float32)) \
            / OUT_DESCALE
    return out
